# revision 17
# baseline (speedup 1.0000x reference)
"""Multi-head attention kernel for 8 TRN2 NeuronCores.

Problem: b=2, n=2048, d=1024, heads=16, hd=64.
  q/k/v = x @ W{q,k,v}.T (+ zero bias)
  per head: softmax(q k^T / sqrt(d)) @ v
  out = concat @ Wo.T (+ zero bias)

Sharding (8 cores): data-parallel over batch (2) x tensor-parallel over
heads (16 heads -> 4 groups of 4). Core c handles batch c//4, heads
4*(c%4) .. 4*(c%4)+3 (feature slice of 256 columns). Wo is applied
row-parallel: each core emits a partial (n, d) output; the host sums the
4 partials per batch. No collectives needed.

All matmuls run in float32r (TF32-like: ~1.5e-4 rel err on a K=1024
contraction, 4x the fp32 rate). Operands feeding f32r matmuls must be
produced "rounded": DMA'd tensors get one DVE conversion pass; on-chip
tensors (Q^T/K^T/V/P^T/out^T) are written as f32r by their producing
copy/activation directly.

Per-core layouts (host pre-transposes so no on-device transposes at all):
  xT  (d, n)   : x[b].T
  wqT/wkT/wvT (d, 256), woT (256, d)
Pipeline:
  QT[feat, n], KT[feat, n]  (PE; contraction over d; f32r out via DVE)
  V[n, feat] + ones column  (PE; natural layout for AV stationary)
  per head h, k-chunk kc (128 k's), q-half sh (1024 q's):
     scores^T[128, 1024] = KT_h^T . QT_h   (PE, K=hd=64, psum)
     P^T = exp(scores^T / 32)              (ACT, psum->sbuf, f32r out)
     avo[65, q] += V_aug^T . P^T           (PE; row 64 = softmax sums)
  normalize: recip(sums) -> partition_broadcast -> mul  (DVE+GPSIMD)
  partial[n, d] = outT^T . woT (PE), DMA out via SBUF.

Biases are structurally zero in this problem spec and are skipped.
"""

import numpy as np

HEADS = 16
D = 1024
N = 2048
B = 2
N_CORES = 8
HPC = HEADS // (N_CORES // B)  # heads per core = 4
HD = D // HEADS                # 64
F = HPC * HD                   # 256 features per core
P = 128


def build_nc(n=N, d=D, hpc=HPC, hd=HD):
    """Build the per-core Bass program (SPMD: same program on all 8 cores)."""
    import concourse.bass as bass
    import concourse.tile as tile
    from concourse import bacc, mybir

    f32 = mybir.dt.float32
    f32r = mybir.dt.float32r
    f = hpc * hd            # per-core feature count (256)
    FC = f // P             # feature chunks (2)
    DC = d // P             # contraction chunks over d (8)
    NT = n // P             # n tiles / k chunks (16)
    QB = min(512, n)        # matmul moving block
    SCW = min(1024, n)      # scores psum width (2 banks)
    NSC = n // SCW          # q-halves
    scale = 1.0 / float(np.sqrt(np.float32(d)))

    nc = bacc.Bacc("TRN2")

    xT = nc.declare_dram_parameter("xT", [d, n], f32, isOutput=False)
    wqT = nc.declare_dram_parameter("wqT", [d, f], f32, isOutput=False)
    wkT = nc.declare_dram_parameter("wkT", [d, f], f32, isOutput=False)
    wvT = nc.declare_dram_parameter("wvT", [d, f], f32, isOutput=False)
    woT = nc.declare_dram_parameter("woT", [f, d], f32, isOutput=False)
    out = nc.declare_dram_parameter("out", [n, d], f32, isOutput=True)

    xT_c = xT.rearrange("(c p) n -> c p n", p=P)
    wqT_c = wqT.rearrange("(c p) f -> c p f", p=P)
    wkT_c = wkT.rearrange("(c p) f -> c p f", p=P)
    wvT_c = wvT.rearrange("(c p) f -> c p f", p=P)
    woT_c = woT.rearrange("(c p) n -> c p n", p=P)

    with tile.TileContext(nc) as tc:
        with (
            tc.tile_pool(name="qkv", bufs=1) as qkv,        # QT/KT/V residents
            tc.tile_pool(name="outT", bufs=1) as outp,
            tc.tile_pool(name="wo", bufs=1) as wop,
            tc.tile_pool(name="pt", bufs=3) as ptp,         # exp(scores^T) tiles
            tc.tile_pool(name="norm", bufs=2) as normp,
        ):
            QT_sb = qkv.tile([P, FC, n], f32r)
            KT_sb = qkv.tile([P, FC, n], f32r)
            V_sb = qkv.tile([P, NT, hpc, hd + 1], f32r)
            outT_sb = outp.tile([P, FC, n], f32r)
            woT_sb = wop.tile([P, FC, d], f32r)
            # ones column of V_aug: memset an f32 const, then write via a
            # rounding DVE copy (direct memset on f32r fails walrus codegen)
            ones_c = wop.tile([P, 1], f32)
            nc.vector.memset(ones_c[:], 1.0)
            nc.vector.tensor_copy(
                V_sb[:, :, :, hd : hd + 1],
                ones_c.to_broadcast([P, NT, hpc, 1]),
            )

            # ---- Phase 0+1: load/convert inputs, projections ----
            with (
                tc.tile_pool(name="xw", bufs=1) as xw,
                tc.tile_pool(name="stage", bufs=3) as stage,
                tc.tile_pool(name="p1ps", bufs=3, space="PSUM") as p1ps,
            ):
                xT_r = xw.tile([P, DC, n], f32r)
                wqT_r = xw.tile([P, DC, f], f32r)
                wkT_r = xw.tile([P, DC, f], f32r)
                wvT_r = xw.tile([P, DC, f], f32r)
                SW = min(1024, n)  # staging chunk width

                def load_round(dst_r, src_ap, w, eng):
                    """DMA f32 -> bounce tile -> rounding copy -> f32r."""
                    for c0 in range(0, w, SW):
                        cw = min(SW, w - c0)
                        st = stage.tile([P, SW], f32, tag="st")
                        nc.sync.dma_start(
                            out=st[:, 0:cw], in_=src_ap[:, c0 : c0 + cw]
                        )
                        eng.tensor_copy(dst_r[:, c0 : c0 + cw], st[:, 0:cw])

                # xT casts on DVE; weight casts on GpSimd so they don't
                # serialize behind the xT casts feeding the first matmuls
                for dc in range(DC):
                    load_round(xT_r[:, dc, :], xT_c[dc], n, nc.vector)
                    load_round(wqT_r[:, dc, :], wqT_c[dc], f, nc.gpsimd)
                    load_round(wkT_r[:, dc, :], wkT_c[dc], f, nc.gpsimd)
                    load_round(wvT_r[:, dc, :], wvT_c[dc], f, nc.gpsimd)
                for fc in range(FC):
                    load_round(woT_sb[:, fc, :], woT_c[fc], d, nc.gpsimd)

                for w_sb, dst in ((wqT_r, QT_sb), (wkT_r, KT_sb)):
                    for fc in range(FC):
                        for qc in range(n // QB):
                            ps = p1ps.tile([P, QB], f32, tag="projps")
                            for dc in range(DC):
                                nc.tensor.matmul(
                                    ps[:],
                                    w_sb[:, dc, fc * P : (fc + 1) * P],
                                    xT_r[:, dc, qc * QB : (qc + 1) * QB],
                                    start=(dc == 0),
                                    stop=(dc == DC - 1),
                                )
                            nc.vector.tensor_copy(
                                dst[:, fc, qc * QB : (qc + 1) * QB], ps[:]
                            )
                for nt in range(NT):
                    ps = p1ps.tile([P, f], f32, tag="projps")
                    for dc in range(DC):
                        nc.tensor.matmul(
                            ps[:],
                            xT_r[:, dc, nt * P : (nt + 1) * P],
                            wvT_r[:, dc, :],
                            start=(dc == 0),
                            stop=(dc == DC - 1),
                        )
                    nc.vector.tensor_copy(
                        V_sb[:, nt, :, 0:hd],
                        ps.rearrange("p (h e) -> p h e", h=hpc),
                    )

            # ---- Phase 2: attention, one (head, q-half) pass at a time ----
            # avps bufs=2 keeps the next pass's AV accumulation running while
            # this pass normalizes, so the PE never idles at pass boundaries
            # (an idle >3.4us drops the HAM clock gate to half speed).
            with (
                tc.tile_pool(name="scps", bufs=2, space="PSUM") as scps,
                tc.tile_pool(name="avps", bufs=2, space="PSUM") as avps,
            ):
                for h in range(hpc):
                    fc = (h * hd) // P
                    po = (h * hd) % P
                    for sh in range(NSC):
                        q0 = sh * SCW
                        avo = avps.tile([hd + 1, SCW], f32, tag="avo")
                        for kc in range(NT):
                            sc = scps.tile([P, SCW], f32, tag="sc")
                            for qc in range(SCW // QB):
                                nc.tensor.matmul(
                                    sc[:, qc * QB : (qc + 1) * QB],
                                    KT_sb[po : po + hd, fc, kc * P : (kc + 1) * P],
                                    QT_sb[
                                        po : po + hd,
                                        fc,
                                        q0 + qc * QB : q0 + (qc + 1) * QB,
                                    ],
                                    start=True,
                                    stop=True,
                                )
                            pt = ptp.tile([P, SCW], f32r, tag="pt")
                            nc.scalar.activation(
                                pt[:], sc[:], mybir.ActivationFunctionType.Exp,
                                scale=scale,
                            )
                            for qc in range(SCW // QB):
                                nc.tensor.matmul(
                                    avo[:, qc * QB : (qc + 1) * QB],
                                    V_sb[:, kc, h, :],
                                    pt[:, qc * QB : (qc + 1) * QB],
                                    start=(kc == 0),
                                    stop=(kc == NT - 1),
                                )
                        # normalize rows 0..hd-1 by row hd (softmax sums)
                        recip = normp.tile([1, SCW], f32, tag="recip")
                        nc.vector.reciprocal(recip[:], avo[hd : hd + 1, :])
                        bc = normp.tile([hd, SCW], f32, tag="bc")
                        nc.gpsimd.partition_broadcast(bc[:], recip[:])
                        nc.vector.tensor_mul(
                            outT_sb[po : po + hd, fc, q0 : q0 + SCW],
                            avo[0:hd, :],
                            bc[:],
                        )

            # ---- Phase 3: output projection (row-parallel partial) ----
            with (
                tc.tile_pool(name="p3ps", bufs=4, space="PSUM") as p3ps,
                tc.tile_pool(name="p3sb", bufs=4) as p3sb,
            ):
                for qt in range(NT):
                    for do in range(d // 512):
                        ps = p3ps.tile([P, 512], f32, tag="wops")
                        for fc in range(FC):
                            nc.tensor.matmul(
                                ps[:],
                                outT_sb[:, fc, qt * P : (qt + 1) * P],
                                woT_sb[:, fc, do * 512 : (do + 1) * 512],
                                start=(fc == 0),
                                stop=(fc == FC - 1),
                            )
                        ob = p3sb.tile([P, 512], f32, tag="wosb")
                        nc.vector.tensor_copy(ob[:], ps[:])
                        nc.sync.dma_start(
                            out=out[qt * P : (qt + 1) * P, do * 512 : (do + 1) * 512],
                            in_=ob[:],
                        )
    nc.finalize()
    return nc


def make_in_maps(x, Wq, Wk, Wv, Wo):
    """Shard full inputs into per-core DRAM parameter maps."""
    x = np.asarray(x, dtype=np.float32)
    Wq = np.asarray(Wq, dtype=np.float32)
    Wk = np.asarray(Wk, dtype=np.float32)
    Wv = np.asarray(Wv, dtype=np.float32)
    Wo = np.asarray(Wo, dtype=np.float32)
    xTs = [np.ascontiguousarray(x[b].T) for b in range(B)]
    WqT, WkT, WvT = Wq.T, Wk.T, Wv.T
    in_maps = []
    for c in range(N_CORES):
        b, g = c // (N_CORES // B), c % (N_CORES // B)
        fs = slice(g * F, (g + 1) * F)
        in_maps.append(
            {
                "xT": xTs[b],
                "wqT": np.ascontiguousarray(WqT[:, fs]),
                "wkT": np.ascontiguousarray(WkT[:, fs]),
                "wvT": np.ascontiguousarray(WvT[:, fs]),
                "woT": np.ascontiguousarray(Wo[:, fs].T),
            }
        )
    return in_maps


_NC_CACHE = {}


def run(x, Wq, Wk, Wv, Wo, trace=False):
    from concourse.bass_utils import run_bass_kernel_spmd

    if "nc" not in _NC_CACHE:
        _NC_CACHE["nc"] = build_nc()
    nc = _NC_CACHE["nc"]
    in_maps = make_in_maps(x, Wq, Wk, Wv, Wo)
    res = run_bass_kernel_spmd(nc, in_maps, core_ids=list(range(N_CORES)), trace=trace)
    parts = [np.asarray(res.results[i]["out"]) for i in range(N_CORES)]
    gpb = N_CORES // B
    full = np.stack(
        [sum(parts[b * gpb + 1 : (b + 1) * gpb], parts[b * gpb]) for b in range(B)]
    )
    return full.astype(np.float32), res


def kernel(x, Wq, bq, Wk, bk, Wv, bv, Wo, bo):
    full, _ = run(x, Wq, Wk, Wv, Wo)
    return full


# revision 20
# speedup vs baseline: 1.3932x; 1.3932x over previous
"""Multi-head attention kernel for 8 TRN2 NeuronCores.

Problem: b=2, n=2048, d=1024, heads=16, hd=64.
  q/k/v = x @ W{q,k,v}.T (+ zero bias)
  per head: softmax(q k^T / sqrt(d)) @ v
  out = concat @ Wo.T (+ zero bias)

Sharding (8 cores): data-parallel over batch (2) x tensor-parallel over
heads (16 heads -> 4 groups of 4). Core c handles batch c//4, heads
4*(c%4) .. 4*(c%4)+3 (feature slice of 256 columns). Wo is applied
row-parallel: each core emits a partial (n, d) output; the host sums the
4 partials per batch. No collectives needed.

All matmuls run in float32r (TF32-like: ~1.5e-4 rel err on a K=1024
contraction, 4x the fp32 rate). Operands feeding f32r matmuls must be
produced "rounded": DMA'd tensors get one DVE conversion pass; on-chip
tensors (Q^T/K^T/V/P^T/out^T) are written as f32r by their producing
copy/activation directly.

Per-core layouts (host pre-transposes so no on-device transposes at all):
  xT  (d, n)   : x[b].T
  wqT/wkT/wvT (d, 256), woT (256, d)
Pipeline:
  QT[feat, n], KT[feat, n]  (PE; contraction over d; f32r out via DVE)
  V[n, feat] + ones column  (PE; natural layout for AV stationary)
  per head h, k-chunk kc (128 k's), q-half sh (1024 q's):
     scores^T[128, 1024] = KT_h^T . QT_h   (PE, K=hd=64, psum)
     P^T = exp(scores^T / 32)              (ACT, psum->sbuf, f32r out)
     avo[65, q] += V_aug^T . P^T           (PE; row 64 = softmax sums)
  normalize: recip(sums) -> partition_broadcast -> mul  (DVE+GPSIMD)
  partial[n, d] = outT^T . woT (PE), DMA out via SBUF.

Biases are structurally zero in this problem spec and are skipped.
"""

import numpy as np

HEADS = 16
D = 1024
N = 2048
B = 2
N_CORES = 8
HPC = HEADS // (N_CORES // B)  # heads per core = 4
HD = D // HEADS                # 64
F = HPC * HD                   # 256 features per core
P = 128


def build_nc(n=N, d=D, hpc=HPC, hd=HD):
    """Build the per-core Bass program (SPMD: same program on all 8 cores)."""
    import concourse.bass as bass
    import concourse.tile as tile
    from concourse import bacc, mybir

    f32 = mybir.dt.float32
    f32r = mybir.dt.float32r
    f = hpc * hd            # per-core feature count (256)
    FC = f // P             # feature chunks (2)
    DC = d // P             # contraction chunks over d (8)
    NT = n // P             # n tiles / k chunks (16)
    QB = min(512, n)        # matmul moving block
    SCW = min(1024, n)      # scores psum width (2 banks)
    NSC = n // SCW          # q-halves
    scale = 1.0 / float(np.sqrt(np.float32(d)))

    nc = bacc.Bacc("TRN2")

    xT = nc.declare_dram_parameter("xT", [d, n], f32, isOutput=False)
    wqT = nc.declare_dram_parameter("wqT", [d, f], f32, isOutput=False)
    wkT = nc.declare_dram_parameter("wkT", [d, f], f32, isOutput=False)
    wvT = nc.declare_dram_parameter("wvT", [d, f], f32, isOutput=False)
    woT = nc.declare_dram_parameter("woT", [f, d], f32, isOutput=False)
    out = nc.declare_dram_parameter("out", [n, d], f32, isOutput=True)

    xT_c = xT.rearrange("(c p) n -> c p n", p=P)
    wqT_c = wqT.rearrange("(c p) f -> c p f", p=P)
    wkT_c = wkT.rearrange("(c p) f -> c p f", p=P)
    wvT_c = wvT.rearrange("(c p) f -> c p f", p=P)
    woT_c = woT.rearrange("(c p) n -> c p n", p=P)

    with tile.TileContext(nc) as tc:
        with (
            tc.tile_pool(name="qkv", bufs=1) as qkv,        # QT/KT/V residents
            tc.tile_pool(name="outT", bufs=1) as outp,
            tc.tile_pool(name="wo", bufs=1) as wop,
        ):
            QT_sb = qkv.tile([P, FC, n], f32r)
            # per-head K^T, zero-padded to a full 128-row stationary: head h
            # occupies partition rows po..po+hd (matching its rows in QT), the
            # other rows are zero.  K=64 matmuls run at 2 cyc/row on HW;
            # zero-padding to K=128 runs at 1 cyc/row for the same math.
            KTz_sb = qkv.tile([P, hpc, n], f32r)
            V_sb = qkv.tile([P, NT, hpc, hd + 1], f32r)
            outT_sb = outp.tile([P, FC, n], f32r)
            woT_sb = wop.tile([P, FC, d], f32r)
            # ones column of V_aug / zero fill of KTz: memset f32 consts, then
            # write via rounding DVE copies (direct memset on f32r fails
            # walrus codegen, and f32r matmul operands need rounding writers)
            ones_c = wop.tile([P, 1], f32)
            nc.vector.memset(ones_c[:], 1.0)
            nc.vector.tensor_copy(
                V_sb[:, :, :, hd : hd + 1],
                ones_c.to_broadcast([P, NT, hpc, 1]),
            )
            zero_c = wop.tile([P, 1], f32)
            nc.vector.memset(zero_c[:], 0.0)
            nc.vector.tensor_copy(
                KTz_sb[:], zero_c.to_broadcast([P, hpc, n])
            )

            # ---- Phase 0+1: load/convert inputs, projections ----
            with (
                tc.tile_pool(name="xw", bufs=1) as xw,
                tc.tile_pool(name="stage", bufs=3) as stage,
                tc.tile_pool(name="p1ps", bufs=3, space="PSUM") as p1ps,
            ):
                xT_r = xw.tile([P, DC, n], f32r)
                wqT_r = xw.tile([P, DC, f], f32r)
                wkT_r = xw.tile([P, DC, f], f32r)
                wvT_r = xw.tile([P, DC, f], f32r)
                SW = min(1024, n)  # staging chunk width

                def load_round(dst_r, src_ap, w, eng):
                    """DMA f32 -> bounce tile -> rounding copy -> f32r."""
                    for c0 in range(0, w, SW):
                        cw = min(SW, w - c0)
                        st = stage.tile([P, SW], f32, tag="st")
                        nc.sync.dma_start(
                            out=st[:, 0:cw], in_=src_ap[:, c0 : c0 + cw]
                        )
                        eng.tensor_copy(dst_r[:, c0 : c0 + cw], st[:, 0:cw])

                # xT casts on DVE; weight casts on GpSimd so they don't
                # serialize behind the xT casts feeding the first matmuls
                for dc in range(DC):
                    load_round(xT_r[:, dc, :], xT_c[dc], n, nc.vector)
                    load_round(wqT_r[:, dc, :], wqT_c[dc], f, nc.gpsimd)
                    load_round(wkT_r[:, dc, :], wkT_c[dc], f, nc.gpsimd)
                    load_round(wvT_r[:, dc, :], wvT_c[dc], f, nc.gpsimd)
                for fc in range(FC):
                    load_round(woT_sb[:, fc, :], woT_c[fc], d, nc.gpsimd)

                for w_sb, is_k in ((wqT_r, False), (wkT_r, True)):
                    for fc in range(FC):
                        for qc in range(n // QB):
                            ps = p1ps.tile([P, QB], f32, tag="projps")
                            for dc in range(DC):
                                nc.tensor.matmul(
                                    ps[:],
                                    w_sb[:, dc, fc * P : (fc + 1) * P],
                                    xT_r[:, dc, qc * QB : (qc + 1) * QB],
                                    start=(dc == 0),
                                    stop=(dc == DC - 1),
                                )
                            sl = slice(qc * QB, (qc + 1) * QB)
                            if is_k:
                                # rows 0:64 = head 2fc (po=0), rows 64:128 =
                                # head 2fc+1 (po=64); keep row alignment
                                nc.vector.tensor_copy(
                                    KTz_sb[0:hd, 2 * fc, sl], ps[0:hd, :]
                                )
                                nc.vector.tensor_copy(
                                    KTz_sb[hd : 2 * hd, 2 * fc + 1, sl],
                                    ps[hd : 2 * hd, :],
                                )
                            else:
                                nc.vector.tensor_copy(QT_sb[:, fc, sl], ps[:])
                for nt in range(NT):
                    ps = p1ps.tile([P, f], f32, tag="projps")
                    for dc in range(DC):
                        nc.tensor.matmul(
                            ps[:],
                            xT_r[:, dc, nt * P : (nt + 1) * P],
                            wvT_r[:, dc, :],
                            start=(dc == 0),
                            stop=(dc == DC - 1),
                        )
                    nc.vector.tensor_copy(
                        V_sb[:, nt, :, 0:hd],
                        ps.rearrange("p (h e) -> p h e", h=hpc),
                    )

            # ---- Phase 2: attention, one (head, q-half) pass at a time ----
            # avps bufs=2 keeps the next pass's AV accumulation running while
            # this pass normalizes, so the PE never idles at pass boundaries
            # (an idle >3.4us drops the HAM clock gate to half speed).
            with (
                tc.tile_pool(name="scps", bufs=2, space="PSUM") as scps,
                tc.tile_pool(name="avps", bufs=2, space="PSUM") as avps,
                tc.tile_pool(name="pt", bufs=3) as ptp,
                tc.tile_pool(name="norm", bufs=2) as normp,
            ):
                for h in range(hpc):
                    fc = (h * hd) // P
                    po = (h * hd) % P
                    for sh in range(NSC):
                        q0 = sh * SCW
                        avo = avps.tile([hd + 1, SCW], f32, tag="avo")
                        for kc in range(NT):
                            sc = scps.tile([P, SCW], f32, tag="sc")
                            for qc in range(SCW // QB):
                                nc.tensor.matmul(
                                    sc[:, qc * QB : (qc + 1) * QB],
                                    KTz_sb[:, h, kc * P : (kc + 1) * P],
                                    QT_sb[
                                        :,
                                        fc,
                                        q0 + qc * QB : q0 + (qc + 1) * QB,
                                    ],
                                    start=True,
                                    stop=True,
                                )
                            pt = ptp.tile([P, SCW], f32r, tag="pt")
                            nc.scalar.activation(
                                pt[:], sc[:], mybir.ActivationFunctionType.Exp,
                                scale=scale,
                            )
                            for qc in range(SCW // QB):
                                nc.tensor.matmul(
                                    avo[:, qc * QB : (qc + 1) * QB],
                                    V_sb[:, kc, h, :],
                                    pt[:, qc * QB : (qc + 1) * QB],
                                    start=(kc == 0),
                                    stop=(kc == NT - 1),
                                )
                        # normalize rows 0..hd-1 by row hd (softmax sums)
                        recip = normp.tile([1, SCW], f32, tag="recip")
                        nc.vector.reciprocal(recip[:], avo[hd : hd + 1, :])
                        bc = normp.tile([hd, SCW], f32, tag="bc")
                        nc.gpsimd.partition_broadcast(bc[:], recip[:])
                        nc.vector.tensor_mul(
                            outT_sb[po : po + hd, fc, q0 : q0 + SCW],
                            avo[0:hd, :],
                            bc[:],
                        )

            # ---- Phase 3: output projection (row-parallel partial) ----
            with (
                tc.tile_pool(name="p3ps", bufs=4, space="PSUM") as p3ps,
                tc.tile_pool(name="p3sb", bufs=4) as p3sb,
            ):
                for qt in range(NT):
                    for do in range(d // 512):
                        ps = p3ps.tile([P, 512], f32, tag="wops")
                        for fc in range(FC):
                            nc.tensor.matmul(
                                ps[:],
                                outT_sb[:, fc, qt * P : (qt + 1) * P],
                                woT_sb[:, fc, do * 512 : (do + 1) * 512],
                                start=(fc == 0),
                                stop=(fc == FC - 1),
                            )
                        ob = p3sb.tile([P, 512], f32, tag="wosb")
                        nc.vector.tensor_copy(ob[:], ps[:])
                        nc.sync.dma_start(
                            out=out[qt * P : (qt + 1) * P, do * 512 : (do + 1) * 512],
                            in_=ob[:],
                        )
    nc.finalize()
    return nc


def make_in_maps(x, Wq, Wk, Wv, Wo):
    """Shard full inputs into per-core DRAM parameter maps."""
    x = np.asarray(x, dtype=np.float32)
    Wq = np.asarray(Wq, dtype=np.float32)
    Wk = np.asarray(Wk, dtype=np.float32)
    Wv = np.asarray(Wv, dtype=np.float32)
    Wo = np.asarray(Wo, dtype=np.float32)
    xTs = [np.ascontiguousarray(x[b].T) for b in range(B)]
    WqT, WkT, WvT = Wq.T, Wk.T, Wv.T
    in_maps = []
    for c in range(N_CORES):
        b, g = c // (N_CORES // B), c % (N_CORES // B)
        fs = slice(g * F, (g + 1) * F)
        in_maps.append(
            {
                "xT": xTs[b],
                "wqT": np.ascontiguousarray(WqT[:, fs]),
                "wkT": np.ascontiguousarray(WkT[:, fs]),
                "wvT": np.ascontiguousarray(WvT[:, fs]),
                "woT": np.ascontiguousarray(Wo[:, fs].T),
            }
        )
    return in_maps


_NC_CACHE = {}


def run(x, Wq, Wk, Wv, Wo, trace=False):
    from concourse.bass_utils import run_bass_kernel_spmd

    if "nc" not in _NC_CACHE:
        _NC_CACHE["nc"] = build_nc()
    nc = _NC_CACHE["nc"]
    in_maps = make_in_maps(x, Wq, Wk, Wv, Wo)
    res = run_bass_kernel_spmd(nc, in_maps, core_ids=list(range(N_CORES)), trace=trace)
    parts = [np.asarray(res.results[i]["out"]) for i in range(N_CORES)]
    gpb = N_CORES // B
    full = np.stack(
        [sum(parts[b * gpb + 1 : (b + 1) * gpb], parts[b * gpb]) for b in range(B)]
    )
    return full.astype(np.float32), res


def kernel(x, Wq, bq, Wk, bk, Wv, bv, Wo, bo):
    full, _ = run(x, Wq, Wk, Wv, Wo)
    return full
